# revision 4
# baseline (speedup 1.0000x reference)
"""KNN memory attention kernel for Trainium2, sharded over the head axis.

Problem shapes (hardcoded): B=2, N=2048, H=8, D=64, K=32.
Each of the 8 NeuronCores handles one head h:
  sim[n,j] = (q[n,:]*exp(scale_h)) . mem_k[n,j,:]        (f32, exact)
  attn     = softmax(sim + mask_bias)  over j
  out[n,:] = local_out[n,:] + sum_j attn[n,j] * mem_v[n,j,:]

Layout on core: rows (b*n) on SBUF partitions (tiles of 128), K*D=2048 on
the free axis.  Stage-1 dot products = broadcast tensor_mul + tensor_reduce
on the Vector engine; exp on the Scalar engine; stage-2 weighted sum =
broadcast mul + strided reduce.
"""

import numpy as np

try:
    import concourse.bass as bass  # noqa: F401
except ImportError:
    import sys

    sys.path.insert(0, "/opt/trn_rl_repo")
    import concourse.bass as bass  # noqa: F401

import concourse.bacc as bacc
import concourse.mybir as mybir
import concourse.tile as tile
from concourse.bass_utils import run_bass_kernel_spmd

B, N, H, D, K = 2, 2048, 8, 64, 32
R = B * N           # rows per core
KD = K * D          # free width of one k/v row
P = 128             # SBUF partitions
NT = R // P         # tiles per core
F32 = mybir.dt.float32
AX = mybir.AxisListType
OP = mybir.AluOpType

_STATE = {}


def _build_nc():
    nc = bacc.Bacc(
        "TRN2", target_bir_lowering=False, debug=False, num_devices=8
    )
    qs = nc.declare_dram_parameter("qs", [R, D], F32, isOutput=False)
    kk = nc.declare_dram_parameter("k", [R, KD], F32, isOutput=False)
    vv = nc.declare_dram_parameter("v", [R, KD], F32, isOutput=False)
    mb = nc.declare_dram_parameter("mb", [R, K], F32, isOutput=False)
    lo = nc.declare_dram_parameter("lo", [R, D], F32, isOutput=False)
    out = nc.declare_dram_parameter("out", [R, D], F32, isOutput=True)

    with tile.TileContext(nc) as tc:
        with (
            tc.tile_pool(name="big", bufs=3) as big,
            tc.tile_pool(name="small", bufs=4) as small,
        ):
            for t in range(NT):
                rows = slice(t * P, (t + 1) * P)

                kt = big.tile([P, KD], F32, tag="kt")
                nc.sync.dma_start(out=kt, in_=kk[rows, :])
                vt = big.tile([P, KD], F32, tag="vt")
                nc.sync.dma_start(out=vt, in_=vv[rows, :])
                qt = small.tile([P, D], F32, tag="qt")
                nc.scalar.dma_start(out=qt, in_=qs[rows, :])
                mbt = small.tile([P, K], F32, tag="mbt")
                nc.scalar.dma_start(out=mbt, in_=mb[rows, :])
                lot = small.tile([P, D], F32, tag="lot")
                nc.scalar.dma_start(out=lot, in_=lo[rows, :])

                # stage 1: sim[p, j] = sum_d k[p, j, d] * q[p, d]
                prod = big.tile([P, KD], F32, tag="prod")
                k3 = kt[:].rearrange("p (j d) -> p j d", j=K)
                q3 = qt[:].unsqueeze(1).broadcast_to((P, K, D))
                nc.vector.tensor_mul(
                    prod[:].rearrange("p (j d) -> p j d", j=K), k3, q3
                )
                sim = small.tile([P, K], F32, tag="sim")
                nc.vector.reduce_sum(
                    sim[:], prod[:].rearrange("p (j d) -> p j d", j=K), axis=AX.X
                )

                # mask + stable softmax over j
                simm = small.tile([P, K], F32, tag="simm")
                nc.vector.tensor_add(simm[:], sim[:], mbt[:])
                nmx = small.tile([P, 1], F32, tag="nmx")
                nc.vector.tensor_reduce(
                    nmx[:], simm[:], axis=AX.X, op=OP.max, negate=True
                )
                e = small.tile([P, K], F32, tag="e")
                nc.scalar.activation(
                    e[:], simm[:], mybir.ActivationFunctionType.Exp, bias=nmx[:]
                )
                ssum = small.tile([P, 1], F32, tag="ssum")
                nc.vector.reduce_sum(ssum[:], e[:], axis=AX.X)
                rr = small.tile([P, 1], F32, tag="rr")
                nc.vector.reciprocal(rr[:], ssum[:])

                # stage 2: mo[p, d] = sum_j e[p, j] * v[p, j, d]
                prod2 = big.tile([P, KD], F32, tag="prod2")
                v3 = vt[:].rearrange("p (j d) -> p j d", j=K)
                e3 = e[:].unsqueeze(2).broadcast_to((P, K, D))
                nc.vector.tensor_mul(
                    prod2[:].rearrange("p (j d) -> p j d", j=K), v3, e3
                )
                mo = small.tile([P, D], F32, tag="mo")
                nc.vector.reduce_sum(
                    mo[:],
                    prod2[:].rearrange("p (j d) -> p j d", j=K).transpose([0, 2, 1]),
                    axis=AX.X,
                )

                # out = mo / ssum + local_out
                ot = small.tile([P, D], F32, tag="ot")
                nc.vector.scalar_tensor_tensor(
                    out=ot[:],
                    in0=mo[:],
                    scalar=rr[:],
                    in1=lot[:],
                    op0=OP.mult,
                    op1=OP.add,
                )
                nc.scalar.dma_start(out=out[rows, :], in_=ot[:])
    nc.finalize()
    return nc


def _get_nc():
    if "nc" not in _STATE:
        _STATE["nc"] = _build_nc()
    return _STATE["nc"]


def _prep_in_maps(inputs):
    q = np.asarray(inputs["q"], dtype=np.float32)
    mem_k = np.asarray(inputs["mem_k"], dtype=np.float32)
    mem_v = np.asarray(inputs["mem_v"], dtype=np.float32)
    mask = np.asarray(inputs["mem_mask"])
    local = np.asarray(inputs["local_out"], dtype=np.float32)
    scale = np.asarray(inputs["scale"], dtype=np.float32)

    s = np.exp(scale.reshape(H))
    q5 = q.reshape(B, N, H, D)
    lo5 = local.reshape(B, N, H, D)
    mbias = np.where(mask, np.float32(0.0), np.float32(-1e30)).astype(np.float32)

    in_maps = []
    for h in range(H):
        in_maps.append(
            {
                "qs": (np.ascontiguousarray(q5[:, :, h, :]) * s[h])
                .reshape(R, D)
                .astype(np.float32),
                "k": np.ascontiguousarray(mem_k[:, h]).reshape(R, KD),
                "v": np.ascontiguousarray(mem_v[:, h]).reshape(R, KD),
                "mb": np.ascontiguousarray(mbias[:, h]).reshape(R, K),
                "lo": np.ascontiguousarray(lo5[:, :, h, :]).reshape(R, D),
            }
        )
    return in_maps


def run(inputs, trace=False):
    """Run on 8 cores; returns (full_output, BassKernelResults)."""
    nc = _get_nc()
    in_maps = _prep_in_maps(inputs)
    res = run_bass_kernel_spmd(nc, in_maps, list(range(8)), trace=trace)
    outs = np.stack([res.results[h]["out"] for h in range(H)])  # (H, R, D)
    full = (
        outs.reshape(H, B, N, D).transpose(1, 2, 0, 3).reshape(B, N, H * D)
    )
    return np.ascontiguousarray(full, dtype=np.float32), res


def kernel(**inputs) -> np.ndarray:
    full, _ = run(inputs, trace=False)
    return full


# revision 8
# speedup vs baseline: 1.1755x; 1.1755x over previous
"""KNN memory attention kernel for Trainium2, sharded over the head axis.

Problem shapes (hardcoded): B=2, N=2048, H=8, D=64, K=32.
Each of the 8 NeuronCores handles one head h:
  sim[n,j] = (q[n,:]*exp(scale_h)) . mem_k[n,j,:]        (f32, exact)
  attn     = softmax(sim + mask_bias)  over j
  mem_out[n,:] = sum_j attn[n,j] * mem_v[n,j,:]
(local_out is added on the host after the gather.)

Per 128-query tile:
  stage 1 (DVE): broadcast tensor_mul + tensor_reduce over d -> sim (128,32).
  softmax: DVE max (negated) -> ACT exp with accumulated sum -> DVE recip
           -> ACT scale by 1/sum -> attn (128,32).
  stage 2 (PE): V is DMA'd as ((4 queries x 32 keys), 64) panels vt.
    - me[q, qq*32+j] = attn[q,j] * [q%4==qq]            (DVE, masked bcast)
    - bdall = me.T @ SEL  (PE; SEL[q, (gg,ch,m)] = [q == 8gg+4ch+(m-4ch)]
      i.e. one-hot at m=q%8, gg=q//8, ch=(q%8)//4) -> (128, 256) PSUM
    - copy bdall -> SBUF (ACT)
    - for gg in 0..16, ch in 0..2: matmul accumulates
        ps[m, 64*gg+d] += sum_(qq,j) bdall[(qq,j), (gg,ch,m)] * vt[(qq,j), 2gg+ch, d]
      giving ps[m, 64*gg+d] = mem_out[t*128 + 8*gg + m, d]
    - DMA ps (PSUM) straight to DRAM with the (gg m d) row remap.
"""

import numpy as np

try:
    import concourse.bass as bass  # noqa: F401
except ImportError:
    import sys

    sys.path.insert(0, "/opt/trn_rl_repo")
    import concourse.bass as bass  # noqa: F401

import concourse.bacc as bacc
import concourse.mybir as mybir
import concourse.tile as tile
from concourse.bass_utils import run_bass_kernel_spmd

B, N, H, D, K = 2, 2048, 8, 64, 32
R = B * N           # rows per core
KD = K * D          # free width of one k/v row
P = 128             # SBUF partitions
NT = R // P         # tiles per core
G = P // 4          # 4-query groups per tile (32)
GG = P // 8         # 8-query pair-groups per tile (16)
F32 = mybir.dt.float32
AX = mybir.AxisListType
OP = mybir.AluOpType

_STATE = {}


def _build_nc():
    nc = bacc.Bacc(
        "TRN2", target_bir_lowering=False, debug=False, num_devices=8
    )
    qs = nc.declare_dram_parameter("qs", [R, D], F32, isOutput=False)
    kk = nc.declare_dram_parameter("k", [R, KD], F32, isOutput=False)
    # v indexed per (query, key) row: row q*K + j holds v[q, j, :]
    vv = nc.declare_dram_parameter("v", [R * K, D], F32, isOutput=False)
    mb = nc.declare_dram_parameter("mb", [R, K], F32, isOutput=False)
    qmask = nc.declare_dram_parameter("qmask", [P, P], F32, isOutput=False)
    sel = nc.declare_dram_parameter("sel", [P, 2 * P], F32, isOutput=False)
    out = nc.declare_dram_parameter("out", [R, D], F32, isOutput=True)

    with tile.TileContext(nc) as tc:
        with (
            tc.tile_pool(name="consts", bufs=1) as consts,
            tc.tile_pool(name="big", bufs=3) as big,
            tc.tile_pool(name="small", bufs=4) as small,
            tc.tile_pool(name="psum", bufs=2, space="PSUM") as psum,
        ):
            qmaskt = consts.tile([P, P], F32, tag="qmaskt")
            nc.sync.dma_start(out=qmaskt, in_=qmask[:, :])
            selt = consts.tile([P, 2 * P], F32, tag="selt")
            nc.sync.dma_start(out=selt, in_=sel[:, :])

            for t in range(NT):
                rows = slice(t * P, (t + 1) * P)

                kt = big.tile([P, KD], F32, tag="kt")
                nc.sync.dma_start(out=kt, in_=kk[rows, :])
                # V panel: partition (qq*32+j), free (g, d);
                # source row (t*128 + 4g + qq)*32 + j == t*4096 + g*128 + p
                vt = big.tile([P, KD], F32, tag="vt")
                nc.sync.dma_start(
                    out=vt[:].rearrange("p (g d) -> p g d", g=G),
                    in_=vv[t * P * K : (t + 1) * P * K, :].rearrange(
                        "(g p) d -> p g d", p=P
                    ),
                )
                qt = small.tile([P, D], F32, tag="qt")
                nc.scalar.dma_start(out=qt, in_=qs[rows, :])
                mbt = small.tile([P, K], F32, tag="mbt")
                nc.scalar.dma_start(out=mbt, in_=mb[rows, :])

                # stage 1: sim[p, j] = sum_d k[p, j, d] * q[p, d]
                prod = big.tile([P, KD], F32, tag="prod")
                k3 = kt[:].rearrange("p (j d) -> p j d", j=K)
                q3 = qt[:].unsqueeze(1).broadcast_to((P, K, D))
                nc.vector.tensor_mul(
                    prod[:].rearrange("p (j d) -> p j d", j=K), k3, q3
                )
                sim = small.tile([P, K], F32, tag="sim")
                nc.vector.reduce_sum(
                    sim[:], prod[:].rearrange("p (j d) -> p j d", j=K), axis=AX.X
                )

                # mask + stable softmax over j
                simm = small.tile([P, K], F32, tag="simm")
                nc.vector.tensor_add(simm[:], sim[:], mbt[:])
                nmx = small.tile([P, 1], F32, tag="nmx")
                nc.vector.tensor_reduce(
                    nmx[:], simm[:], axis=AX.X, op=OP.max, negate=True
                )
                e = small.tile([P, K], F32, tag="e")
                ssum = small.tile([P, 1], F32, tag="ssum")
                nc.scalar.activation(
                    e[:],
                    simm[:],
                    mybir.ActivationFunctionType.Exp,
                    bias=nmx[:],
                    accum_out=ssum[:],
                )
                rr = small.tile([P, 1], F32, tag="rr")
                nc.vector.reciprocal(rr[:], ssum[:])
                attn = small.tile([P, K], F32, tag="attn")
                nc.scalar.mul(attn[:], e[:], rr[:])

                # me[q, qq*32+j] = attn[q, j] * qmask[q, qq*32+j]
                me = small.tile([P, P], F32, tag="me")
                me3 = me[:].rearrange("p (c j) -> p c j", c=4)
                nc.vector.tensor_mul(
                    me3,
                    attn[:].unsqueeze(1).broadcast_to((P, 4, K)),
                    qmaskt[:].rearrange("p (c j) -> p c j", c=4),
                )

                # bdall[(qq,j), f] = sum_q me[q, (qq,j)] * sel[q, f]
                bd_ps = psum.tile([P, 2 * P], F32, tag="bd_ps")
                nc.tensor.matmul(
                    bd_ps[:], me[:], selt[:], start=True, stop=True
                )
                bd = small.tile([P, 2 * P], F32, tag="bd")
                nc.scalar.copy(bd[:], bd_ps[:])

                # stage 2: 32 accumulating matmuls into (8, 1024) PSUM
                ps = psum.tile([8, GG * D], F32, tag="ps")
                vt3 = vt[:].rearrange("p (g d) -> p g d", g=G)
                for gg in range(GG):
                    for ch in range(2):
                        nc.tensor.matmul(
                            ps[:, gg * D : (gg + 1) * D],
                            bd[:, (gg * 2 + ch) * 8 : (gg * 2 + ch + 1) * 8],
                            vt3[:, 2 * gg + ch, :],
                            start=(ch == 0),
                            stop=(ch == 1),
                        )

                # PSUM -> SBUF, then DRAM with row remap: out[t*128+8*gg+m, d]
                ot = small.tile([8, GG * D], F32, tag="ot")
                nc.scalar.copy(ot[:], ps[:])
                nc.scalar.dma_start(
                    out=out[rows, :].rearrange("(gg m) d -> m gg d", m=8),
                    in_=ot[:].rearrange("m (gg d) -> m gg d", gg=GG),
                )
    nc.finalize()
    return nc


def _get_nc():
    if "nc" not in _STATE:
        _STATE["nc"] = _build_nc()
    return _STATE["nc"]


def _make_consts():
    qmask = np.zeros((P, P), dtype=np.float32)
    for q in range(P):
        qq = q % 4
        qmask[q, qq * 32 : (qq + 1) * 32] = 1.0
    sel = np.zeros((P, 2 * P), dtype=np.float32)
    for q in range(P):
        gg, m = q // 8, q % 8
        ch = (q % 8) // 4
        sel[q, (gg * 2 + ch) * 8 + m] = 1.0
    return qmask, sel


def _prep_in_maps(inputs):
    q = np.asarray(inputs["q"], dtype=np.float32)
    mem_k = np.asarray(inputs["mem_k"], dtype=np.float32)
    mem_v = np.asarray(inputs["mem_v"], dtype=np.float32)
    mask = np.asarray(inputs["mem_mask"])
    scale = np.asarray(inputs["scale"], dtype=np.float32)

    s = np.exp(scale.reshape(H))
    q5 = q.reshape(B, N, H, D)
    mbias = np.where(mask, np.float32(0.0), np.float32(-1e30)).astype(np.float32)
    qmask, sel = _make_consts()

    in_maps = []
    for h in range(H):
        in_maps.append(
            {
                "qs": (np.ascontiguousarray(q5[:, :, h, :]) * s[h])
                .reshape(R, D)
                .astype(np.float32),
                "k": np.ascontiguousarray(mem_k[:, h]).reshape(R, KD),
                "v": np.ascontiguousarray(mem_v[:, h]).reshape(R * K, D),
                "mb": np.ascontiguousarray(mbias[:, h]).reshape(R, K),
                "qmask": qmask,
                "sel": sel,
            }
        )
    return in_maps


def run(inputs, trace=False):
    """Run on 8 cores; returns (full_output, BassKernelResults)."""
    nc = _get_nc()
    in_maps = _prep_in_maps(inputs)
    res = run_bass_kernel_spmd(nc, in_maps, list(range(8)), trace=trace)
    outs = np.stack([res.results[h]["out"] for h in range(H)])  # (H, R, D)
    full = (
        outs.reshape(H, B, N, D).transpose(1, 2, 0, 3).reshape(B, N, H * D)
    )
    full = full + np.asarray(inputs["local_out"], dtype=np.float32)
    return np.ascontiguousarray(full, dtype=np.float32), res


def kernel(**inputs) -> np.ndarray:
    full, _ = run(inputs, trace=False)
    return full


# revision 12
# speedup vs baseline: 1.3755x; 1.1701x over previous
"""KNN memory attention kernel for Trainium2, sharded over the head axis.

Problem shapes (hardcoded): B=2, N=2048, H=8, D=64, K=32.
Each of the 8 NeuronCores handles one head h:
  sim[n,j] = (q[n,:]*exp(scale_h)) . mem_k[n,j,:]        (f32, exact)
  attn     = softmax(sim + mask_bias)  over j
  mem_out[n,:] = sum_j attn[n,j] * mem_v[n,j,:]
(local_out is added on the host after the gather.)

Per 128-query tile:
  stage 1 (DVE): broadcast tensor_mul + tensor_reduce over d -> sim (128,32).
  softmax: DVE max (negated) -> ACT exp with accumulated sum -> DVE recip
           -> ACT scale by 1/sum -> attn (128,32).
  stage 2 (PE): V is DMA'd as ((4 queries x 32 keys), 64) panels vt.
    - me[q, qq*32+j] = attn[q,j] * [q%4==qq]            (DVE, masked bcast)
    - bdall = me.T @ SEL  (PE; SEL[q, (gg,ch,m)] = [q == 8gg+4ch+(m-4ch)]
      i.e. one-hot at m=q%8, gg=q//8, ch=(q%8)//4) -> (128, 256) PSUM
    - copy bdall -> SBUF (ACT)
    - for gg in 0..16, ch in 0..2: matmul accumulates
        ps[m, 64*gg+d] += sum_(qq,j) bdall[(qq,j), (gg,ch,m)] * vt[(qq,j), 2gg+ch, d]
      giving ps[m, 64*gg+d] = mem_out[t*128 + 8*gg + m, d]
    - DMA ps (PSUM) straight to DRAM with the (gg m d) row remap.
"""

import numpy as np

try:
    import concourse.bass as bass  # noqa: F401
except ImportError:
    import sys

    sys.path.insert(0, "/opt/trn_rl_repo")
    import concourse.bass as bass  # noqa: F401

import concourse.bacc as bacc
import concourse.mybir as mybir
import concourse.tile as tile
from concourse.bass_utils import run_bass_kernel_spmd

B, N, H, D, K = 2, 2048, 8, 64, 32
R = B * N           # rows per core
KD = K * D          # free width of one k/v row
P = 128             # SBUF partitions
NT = R // P         # tiles per core
G = P // 4          # 4-query groups per tile (32)
GG = P // 8         # 8-query pair-groups per tile (16)
F32 = mybir.dt.float32
AX = mybir.AxisListType
OP = mybir.AluOpType

_STATE = {}


def _build_nc():
    nc = bacc.Bacc(
        "TRN2", target_bir_lowering=False, debug=False, num_devices=8
    )
    qs = nc.declare_dram_parameter("qs", [R, D], F32, isOutput=False)
    kk = nc.declare_dram_parameter("k", [R, KD], F32, isOutput=False)
    # v pre-arranged on host into per-tile panels: row (t*128 + p) holds
    # the 32 groups' d-vectors for partition p = (qq*32 + j) of tile t.
    vv = nc.declare_dram_parameter("v", [R, KD], F32, isOutput=False)
    mb = nc.declare_dram_parameter("mb", [R, K], F32, isOutput=False)
    qmask = nc.declare_dram_parameter("qmask", [P, P], F32, isOutput=False)
    sel = nc.declare_dram_parameter("sel", [P, 2 * P], F32, isOutput=False)
    out = nc.declare_dram_parameter("out", [R, D], F32, isOutput=True)

    with tile.TileContext(nc) as tc:
        with (
            tc.tile_pool(name="consts", bufs=1) as consts,
            tc.tile_pool(name="big", bufs=3) as big,
            tc.tile_pool(name="small", bufs=4) as small,
            tc.tile_pool(name="psum", bufs=2, space="PSUM") as psum,
        ):
            qmaskt = consts.tile([P, P], F32, tag="qmaskt")
            nc.sync.dma_start(out=qmaskt, in_=qmask[:, :])
            selt = consts.tile([P, 2 * P], F32, tag="selt")
            nc.sync.dma_start(out=selt, in_=sel[:, :])

            for t in range(NT):
                rows = slice(t * P, (t + 1) * P)

                kt = big.tile([P, KD], F32, tag="kt")
                nc.sync.dma_start(out=kt, in_=kk[rows, :])
                # V panel: partition (qq*32+j), free (g, d);
                # source row (t*128 + 4g + qq)*32 + j == t*4096 + g*128 + p
                vt = big.tile([P, KD], F32, tag="vt")
                nc.sync.dma_start(out=vt, in_=vv[rows, :])
                qt = small.tile([P, D], F32, tag="qt")
                nc.scalar.dma_start(out=qt, in_=qs[rows, :])
                mbt = small.tile([P, K], F32, tag="mbt")
                nc.scalar.dma_start(out=mbt, in_=mb[rows, :])

                # stage 1: sim[p, j] = sum_d k[p, j, d] * q[p, d]
                prod = big.tile([P, KD], F32, tag="prod")
                k3 = kt[:].rearrange("p (j d) -> p j d", j=K)
                q3 = qt[:].unsqueeze(1).broadcast_to((P, K, D))
                nc.vector.tensor_mul(
                    prod[:].rearrange("p (j d) -> p j d", j=K), k3, q3
                )
                sim = small.tile([P, K], F32, tag="sim")
                nc.vector.reduce_sum(
                    sim[:], prod[:].rearrange("p (j d) -> p j d", j=K), axis=AX.X
                )

                # mask + stable softmax over j
                simm = small.tile([P, K], F32, tag="simm")
                nc.vector.tensor_add(simm[:], sim[:], mbt[:])
                nmx = small.tile([P, 1], F32, tag="nmx")
                nc.vector.tensor_reduce(
                    nmx[:], simm[:], axis=AX.X, op=OP.max, negate=True
                )
                e = small.tile([P, K], F32, tag="e")
                ssum = small.tile([P, 1], F32, tag="ssum")
                nc.scalar.activation(
                    e[:],
                    simm[:],
                    mybir.ActivationFunctionType.Exp,
                    bias=nmx[:],
                    accum_out=ssum[:],
                )
                rr = small.tile([P, 1], F32, tag="rr")
                nc.vector.reciprocal(rr[:], ssum[:])
                attn = small.tile([P, K], F32, tag="attn")
                nc.scalar.mul(attn[:], e[:], rr[:])

                # me[q, qq*32+j] = attn[q, j] * qmask[q, qq*32+j]
                me = small.tile([P, P], F32, tag="me")
                me3 = me[:].rearrange("p (c j) -> p c j", c=4)
                nc.vector.tensor_mul(
                    me3,
                    attn[:].unsqueeze(1).broadcast_to((P, 4, K)),
                    qmaskt[:].rearrange("p (c j) -> p c j", c=4),
                )

                # bdall[(qq,j), f] = sum_q me[q, (qq,j)] * sel[q, f]
                bd_ps = psum.tile([P, 2 * P], F32, tag="bd_ps")
                nc.tensor.matmul(
                    bd_ps[:], me[:], selt[:], start=True, stop=True
                )
                bd = small.tile([P, 2 * P], F32, tag="bd")
                nc.scalar.copy(bd[:], bd_ps[:])

                # stage 2: 32 accumulating matmuls into (8, 1024) PSUM
                ps = psum.tile([8, GG * D], F32, tag="ps")
                vt3 = vt[:].rearrange("p (g d) -> p g d", g=G)
                for gg in range(GG):
                    for ch in range(2):
                        nc.tensor.matmul(
                            ps[:, gg * D : (gg + 1) * D],
                            bd[:, (gg * 2 + ch) * 8 : (gg * 2 + ch + 1) * 8],
                            vt3[:, 2 * gg + ch, :],
                            start=(ch == 0),
                            stop=(ch == 1),
                        )

                # PSUM -> SBUF, then DRAM with row remap: out[t*128+8*gg+m, d]
                ot = small.tile([8, GG * D], F32, tag="ot")
                nc.scalar.copy(ot[:], ps[:])
                nc.scalar.dma_start(
                    out=out[rows, :].rearrange("(gg m) d -> m gg d", m=8),
                    in_=ot[:].rearrange("m (gg d) -> m gg d", gg=GG),
                )
    nc.finalize()
    return nc


def _get_nc():
    if "nc" not in _STATE:
        _STATE["nc"] = _build_nc()
    return _STATE["nc"]


def _make_consts():
    qmask = np.zeros((P, P), dtype=np.float32)
    for q in range(P):
        qq = q % 4
        qmask[q, qq * 32 : (qq + 1) * 32] = 1.0
    sel = np.zeros((P, 2 * P), dtype=np.float32)
    for q in range(P):
        gg, m = q // 8, q % 8
        ch = (q % 8) // 4
        sel[q, (gg * 2 + ch) * 8 + m] = 1.0
    return qmask, sel


def _panelize_v(v_h):
    """(B, N, K, D) head-slice -> (R, K*D) panel rows: row (t*128 + qq*32+j)
    holds [v[t*128+4g+qq, j, :] for g in 0..32]."""
    v4 = np.ascontiguousarray(v_h).reshape(NT, G, 4, K, D)  # (t, g, qq, j, d)
    v_re = v4.transpose(0, 2, 3, 1, 4)  # (t, qq, j, g, d)
    return np.ascontiguousarray(v_re).reshape(R, KD)


def _prep_in_maps(inputs):
    q = np.asarray(inputs["q"], dtype=np.float32)
    mem_k = np.asarray(inputs["mem_k"], dtype=np.float32)
    mem_v = np.asarray(inputs["mem_v"], dtype=np.float32)
    mask = np.asarray(inputs["mem_mask"])
    scale = np.asarray(inputs["scale"], dtype=np.float32)

    s = np.exp(scale.reshape(H))
    q5 = q.reshape(B, N, H, D)
    mbias = np.where(mask, np.float32(0.0), np.float32(-1e30)).astype(np.float32)
    qmask, sel = _make_consts()

    in_maps = []
    for h in range(H):
        in_maps.append(
            {
                "qs": (np.ascontiguousarray(q5[:, :, h, :]) * s[h])
                .reshape(R, D)
                .astype(np.float32),
                "k": np.ascontiguousarray(mem_k[:, h]).reshape(R, KD),
                "v": _panelize_v(mem_v[:, h]),
                "mb": np.ascontiguousarray(mbias[:, h]).reshape(R, K),
                "qmask": qmask,
                "sel": sel,
            }
        )
    return in_maps


def run(inputs, trace=False):
    """Run on 8 cores; returns (full_output, BassKernelResults)."""
    nc = _get_nc()
    in_maps = _prep_in_maps(inputs)
    res = run_bass_kernel_spmd(nc, in_maps, list(range(8)), trace=trace)
    outs = np.stack([res.results[h]["out"] for h in range(H)])  # (H, R, D)
    full = (
        outs.reshape(H, B, N, D).transpose(1, 2, 0, 3).reshape(B, N, H * D)
    )
    full = full + np.asarray(inputs["local_out"], dtype=np.float32)
    return np.ascontiguousarray(full, dtype=np.float32), res


def kernel(**inputs) -> np.ndarray:
    full, _ = run(inputs, trace=False)
    return full
